# revision 1
# baseline (speedup 1.0000x reference)
"""Trainium2 Bass kernel for nn_Bottleneck (BN -> 1x1conv -> ReLU -> BN -> 1x1conv).

Strategy: data-parallel over batch (32 -> 4 per core x 8 cores).
- BN folds into the following 1x1 conv: bn(x) = s*x + t per channel, so
  conv(bn(x)) = (W diag(s)) x + (W t + b). Stats (mean, E[x^2]) are computed
  per-shard with DVE bn_stats and globalized with small AllReduces.
- x streams HBM->SBUF once via SWDGE casting DMAs (fp32->bf16 in the DMA
  datapath), staying SBUF-resident for the conv pass. bn_stats runs on the
  bf16 tiles (quantization effect on stats is ~1e-5 relative).
- The collective stream is primed with a dummy AllReduce at kernel start;
  stats AllReduces are split (chunks 0-5 / 6-7 for BN1, per-m-half for BN2)
  so most collective latency hides under the load / conv phases.
- conv1 is ordered m-outer with k innermost across small PSUM stages so each
  LDWEIGHTS covers multiple matmuls and BN2 stats for the m=0 half complete
  at conv1 halftime.
"""
import sys

sys.path.insert(0, "/opt/trn_rl_repo")

import numpy as np

import concourse.bass as bass
import concourse.bacc as bacc
import concourse.mybir as mybir
import concourse.tile as tile
from concourse import bass_utils

# Problem shapes (hardcoded per contract)
B_FULL = 32
N_CORES = 8
B = B_FULL // N_CORES  # 4 batches per core
C1 = 1024  # in channels
C2 = 256   # mid channels
C3 = 64    # out channels
T = 2048   # sequence length
P = 128    # partitions
K1 = C1 // P  # 8 contraction chunks for conv1
K2 = C2 // P  # 2 contraction chunks for conv2
NT = T // 512  # 4 tiles of 512 along T
EPS = 1e-5

AR1_SPLIT = 6   # chunks [0,AR1_SPLIT) in the first (hidden) AllReduce
N_FILLER = 22   # PE warmup matmuls bridging the ARb wait

F32 = mybir.dt.float32
BF16 = mybir.dt.bfloat16
AF = mybir.ActivationFunctionType
ALU = mybir.AluOpType


def _rsqrt(nc, pool, out, v, tmp_tag):
    """out = 1/sqrt(v) fp32 via DVE reciprocal + ACT sqrt (ACT Rsqrt banned)."""
    shape = [P, v.shape[1]]
    r = pool.tile(shape, F32, tag=tmp_tag + "_r", name=tmp_tag + "_r")
    nc.vector.reciprocal(r[:], v)
    nc.scalar.activation(out, r[:], AF.Sqrt)


def build():
    nc = bacc.Bacc("TRN2", target_bir_lowering=False, debug=False,
                   num_devices=N_CORES, num_swdge_queues=4)

    x_d = nc.dram_tensor("x", [B, C1, T], F32, kind="ExternalInput")
    w1t_d = nc.dram_tensor("w1t", [C1, C2], F32, kind="ExternalInput")
    w2t_d = nc.dram_tensor("w2t", [C2, C3], F32, kind="ExternalInput")
    g1_d = nc.dram_tensor("g1", [C1], F32, kind="ExternalInput")
    b1bn_d = nc.dram_tensor("b1bn", [C1], F32, kind="ExternalInput")
    b1c_d = nc.dram_tensor("b1c", [C2], F32, kind="ExternalInput")
    g2_d = nc.dram_tensor("g2", [C2], F32, kind="ExternalInput")
    b2bn_d = nc.dram_tensor("b2bn", [C2], F32, kind="ExternalInput")
    b2c_d = nc.dram_tensor("b2c", [C3], F32, kind="ExternalInput")
    out_d = nc.dram_tensor("out", [B, C3, T], F32, kind="ExternalOutput")

    rg = [list(range(N_CORES))]

    with tile.TileContext(nc) as tc:
        with (
            tc.tile_pool(name="const", bufs=1) as cst,
            tc.tile_pool(name="xpool", bufs=1) as xp,
            tc.tile_pool(name="hpool", bufs=1) as hp,
            tc.tile_pool(name="opool", bufs=3) as op,
            tc.tile_pool(name="ps1", bufs=2, space="PSUM") as ps1,
            tc.tile_pool(name="psmv", bufs=1, space="PSUM") as psmv,
            tc.tile_pool(name="ps2", bufs=3, space="PSUM") as ps2,
            tc.tile_pool(name="dram", bufs=1, space="DRAM") as dram,
        ):
            # ---- prefetch weights/vectors ----
            w1u = cst.tile([P, K1, C2], BF16, tag="w1u")   # W1^T, unscaled bf16
            nc.gpsimd.dma_start(w1u[:], w1t_d.ap().rearrange("(k p) o -> p k o", p=P))
            w2u = cst.tile([P, K2, C3], BF16, tag="w2u")
            nc.gpsimd.dma_start(w2u[:], w2t_d.ap().rearrange("(k p) o -> p k o", p=P))

            g1 = cst.tile([P, K1], F32, tag="g1")
            nc.gpsimd.dma_start(g1[:], g1_d.ap().rearrange("(k p) -> p k", p=P))
            b1bn = cst.tile([P, K1], F32, tag="b1bn")
            nc.gpsimd.dma_start(b1bn[:], b1bn_d.ap().rearrange("(k p) -> p k", p=P))
            b1c = cst.tile([P, K2], F32, tag="b1c")
            nc.gpsimd.dma_start(b1c[:], b1c_d.ap().rearrange("(m p) -> p m", p=P))
            g2 = cst.tile([P, K2], F32, tag="g2")
            nc.gpsimd.dma_start(g2[:], g2_d.ap().rearrange("(k p) -> p k", p=P))
            b2bn = cst.tile([P, K2], F32, tag="b2bn")
            nc.gpsimd.dma_start(b2bn[:], b2bn_d.ap().rearrange("(k p) -> p k", p=P))
            b2c = cst.tile([P, 1], F32, tag="b2c")
            nc.gpsimd.dma_start(b2c[0:C3, :],
                              b2c_d.ap().rearrange("(a o) -> o a", a=1))
            nc.gpsimd.dma_start(b2c[C3:2 * C3, :],
                              b2c_d.ap().rearrange("(a o) -> o a", a=1))

            # ---- prime the collective stream mid-load (tiny payload, its
            # bandwidth steal is negligible; absorbs the ~28-48us first-
            # collective setup so the real stats AR runs at ~26us flat) ----
            pr_in = dram.tile([P, 2], F32, tag="pr_in")
            pr_out = dram.tile([P, 2], F32, tag="pr_out")
            nc.gpsimd.dma_start(pr_in[:], g1[:, 0:2])
            nc.gpsimd.collective_compute(
                "AllReduce", ALU.add, replica_groups=rg,
                ins=[pr_in.opt()], outs=[pr_out.opt()])

            # ---- pass 1: SWDGE casting DMAs stream x fp32->bf16; DVE
            # bn_stats per (k, b) row; per-chunk aggregation ----
            x_bf = [xp.tile([P, K1, T], BF16, tag=f"x_{b}", name=f"x_{b}")
                    for b in range(B)]
            stx = [cst.tile([P, B * NT, 6], F32, tag=f"stx_{k}", name=f"stx_{k}")
                   for k in range(K1)]
            mv1 = cst.tile([P, K1, 2], F32, tag="mv1")
            ar1 = cst.tile([P, K1, 2], F32, tag="ar1")
            tmp1 = cst.tile([P, K1], F32, tag="tmp1")
            arr1 = cst.tile([P, K1, 2], F32, tag="arr1")

            # Four 2-chunk AllReduces pipelined against the load. Placement
            # rules learned from traces: (a) a gpsimd-dispatched op emitted
            # between x-loads stalls SWDGE descriptor gen on its upstream
            # dep, so each span's AR input DMA + collective are emitted only
            # once their DVE pack is already done (2-chunk lag); (b) a
            # collective's ring position is behind all x-DMAs emitted before
            # it, so early spans' collectives fire mid-load; (c) the first
            # collective after the startup barrier costs ~28us flat (stream
            # setup) regardless of payload, so span (0,2) self-primes while
            # the load streams.
            # ONE post-load stats AllReduce: measured across v0/v2/v5/v10,
            # the first stats collective cannot start before ~load-end (its
            # SDMA activity and the load's are mutually degrading), so
            # splitting it only adds serial collective latency. A single
            # 2048-elem AR (~26us primed) beats ARa+ARb (~27+8+gaps).
            spans = [(0, K1)]
            ar_outs = []

            def emit_pack(k0, k1):
                sl = slice(k0, k1)
                nc.vector.tensor_copy(ar1[:, sl, 0], mv1[:, sl, 0])
                nc.vector.tensor_mul(tmp1[:, sl], mv1[:, sl, 0], mv1[:, sl, 0])
                nc.vector.tensor_add(ar1[:, sl, 1], mv1[:, sl, 1], tmp1[:, sl])

            def emit_cc(k0, k1):
                sl = slice(k0, k1)
                ai = dram.tile([P, (k1 - k0) * 2], F32, tag=f"ar1_in_{k0}",
                               name=f"ar1_in_{k0}")
                ao = dram.tile([P, (k1 - k0) * 2], F32, tag=f"ar1_out_{k0}",
                               name=f"ar1_out_{k0}")
                nc.gpsimd.dma_start(ai[:], ar1[:, sl, :])
                nc.gpsimd.collective_compute(
                    "AllReduce", ALU.add, replica_groups=rg,
                    ins=[ai.opt()], outs=[ao.opt()])
                ar_outs.append((sl, ao))

            cc_after_dmas = {}
            pack_after_stats = {K1 - 1: (0, K1)}
            for k in range(K1):
                for b in range(B):
                    nc.gpsimd.dma_start(
                        x_bf[b][:, k, :], x_d[b, k * P:(k + 1) * P, :])
                if k in cc_after_dmas:
                    emit_cc(*cc_after_dmas[k])
                for b in range(B):
                    for c in range(NT):
                        nc.vector.bn_stats(
                            stx[k][:, b * NT + c, :],
                            x_bf[b][:, k, c * 512:(c + 1) * 512])
                nc.vector.bn_aggr(mv1[:, k, :], stx[k][:])
                if k in pack_after_stats:
                    emit_pack(*pack_after_stats[k])
            emit_cc(0, K1)

            # ---- global BN1 affine per span: s1 = g/sqrt(var+eps),
            # t1 = b - mean*s1; fold scale into conv1 weights ----
            mean1 = cst.tile([P, K1], F32, tag="mean1")
            var1 = cst.tile([P, K1], F32, tag="var1")
            rs1 = cst.tile([P, K1], F32, tag="rs1")
            s1 = cst.tile([P, K1], F32, tag="s1")
            t1 = cst.tile([P, K1], F32, tag="t1")
            t1b = cst.tile([P, K1], BF16, tag="t1b")
            w1s = cst.tile([P, K1, C2], BF16, tag="w1s")
            for (sl, ao) in ar_outs:
                nc.sync.dma_start(arr1[:, sl, :], ao[:])
                nc.vector.tensor_scalar_mul(mean1[:, sl], arr1[:, sl, 0],
                                            1.0 / N_CORES)
                nc.vector.tensor_scalar_mul(var1[:, sl], arr1[:, sl, 1],
                                            1.0 / N_CORES)
                nc.vector.tensor_mul(tmp1[:, sl], mean1[:, sl], mean1[:, sl])
                nc.vector.tensor_sub(var1[:, sl], var1[:, sl], tmp1[:, sl])
                nc.vector.tensor_scalar_add(var1[:, sl], var1[:, sl], EPS)
                _rsqrt(nc, cst, rs1[:, sl], var1[:, sl], f"nr1_{sl.start}")
                nc.vector.tensor_mul(s1[:, sl], rs1[:, sl], g1[:, sl])
                nc.vector.tensor_mul(t1[:, sl], mean1[:, sl], s1[:, sl])
                nc.vector.tensor_sub(t1[:, sl], b1bn[:, sl], t1[:, sl])
                nc.vector.tensor_copy(t1b[:, sl], t1[:, sl])
                for k in range(sl.start, sl.stop):
                    nc.vector.tensor_scalar_mul(w1s[:, k, :], w1u[:, k, :],
                                                s1[:, k:k + 1])

            # ---- PE warmup fillers: gated on the last x chunk so they run
            # during the ARb wait, right before conv1 ----
            fps = ps1.tile([P, 512], F32, tag="c1a", name="filler_ps")
            for i in range(N_FILLER):
                nc.tensor.matmul(fps[:], w1u[:, K1 - 1, 0:P],
                                 x_bf[B - 1][:, K1 - 1, 0:512],
                                 start=(i == 0), stop=(i == N_FILLER - 1))

            # effective bias b1' = W1 @ t1 + b1 (PE matvec, unscaled weights)
            b1f = cst.tile([P, K2], F32, tag="b1f")
            for m in range(K2):
                pm = psmv.tile([P, 1], F32, tag="mv", name=f"pm_{m}")
                for k in range(K1):
                    nc.tensor.matmul(pm[:], w1u[:, k, m * P:(m + 1) * P],
                                     t1b[:, k:k + 1],
                                     start=(k == 0), stop=(k == K1 - 1))
                nc.vector.tensor_add(b1f[:, m:m + 1], pm[:], b1c[:, m:m + 1])

            # ---- conv1 (+ReLU) -> h bf16, m-outer; BN2 stats per h tile;
            # AR2 split per m-half so the m=0 collective hides under m=1 ----
            h_bf = [[hp.tile([P, T], BF16, tag=f"h_{m}_{b}", name=f"h_{m}_{b}")
                     for b in range(B)] for m in range(K2)]
            sth = [cst.tile([P, B * NT, 6], F32, tag=f"sth_{m}", name=f"sth_{m}")
                   for m in range(K2)]
            mv2 = cst.tile([P, K2, 2], F32, tag="mv2")
            ar2 = cst.tile([P, K2, 2], F32, tag="ar2")
            tmp2 = cst.tile([P, K2], F32, tag="tmp2")
            ar2_outs = []
            for m in range(K2):
                for tq in range(NT):
                    for bp in range(B // 2):
                        pss = []
                        for hb in range(2):
                            b = 2 * bp + hb
                            ps = ps1.tile([P, 512], F32, tag=f"c1{'ab'[hb]}",
                                          name=f"ps1_{m}_{tq}_{b}")
                            pss.append((b, ps))
                        for k in range(K1):
                            for (b, ps) in pss:
                                nc.tensor.matmul(
                                    ps[:], w1s[:, k, m * P:(m + 1) * P],
                                    x_bf[b][:, k, tq * 512:(tq + 1) * 512],
                                    start=(k == 0), stop=(k == K1 - 1))
                        for (b, ps) in pss:
                            nc.scalar.activation(
                                h_bf[m][b][:, tq * 512:(tq + 1) * 512],
                                ps[:], AF.Relu, bias=b1f[:, m:m + 1])
                            nc.vector.bn_stats(
                                sth[m][:, b * NT + tq, :],
                                h_bf[m][b][:, tq * 512:(tq + 1) * 512])
                # aggregate + pack + AllReduce for this m-half
                nc.vector.bn_aggr(mv2[:, m, :], sth[m][:])
                ms = slice(m, m + 1)
                nc.vector.tensor_copy(ar2[:, ms, 0], mv2[:, ms, 0])
                nc.vector.tensor_mul(tmp2[:, ms], mv2[:, ms, 0], mv2[:, ms, 0])
                nc.vector.tensor_add(ar2[:, ms, 1], mv2[:, ms, 1], tmp2[:, ms])
                ai2 = dram.tile([P, 2], F32, tag=f"ar2_in_{m}",
                                name=f"ar2_in_{m}")
                ao2 = dram.tile([P, 2], F32, tag=f"ar2_out_{m}",
                                name=f"ar2_out_{m}")
                nc.gpsimd.dma_start(ai2[:], ar2[:, ms, :])
                nc.gpsimd.collective_compute(
                    "AllReduce", ALU.add, replica_groups=rg,
                    ins=[ai2.opt()], outs=[ao2.opt()])
                ar2_outs.append(ao2)

            # ---- global BN2 affine + fold into conv2 ----
            arr2 = cst.tile([P, K2, 2], F32, tag="arr2")
            mean2 = cst.tile([P, K2], F32, tag="mean2")
            var2 = cst.tile([P, K2], F32, tag="var2")
            rs2 = cst.tile([P, K2], F32, tag="rs2")
            s2 = cst.tile([P, K2], F32, tag="s2")
            t2 = cst.tile([P, K2], F32, tag="t2")
            t2b = cst.tile([P, K2], BF16, tag="t2b")
            w2s = cst.tile([P, K2, C3], BF16, tag="w2s")
            for m in range(K2):
                sl = slice(m, m + 1)
                nc.sync.dma_start(arr2[:, sl, :], ar2_outs[m][:])
                nc.vector.tensor_scalar_mul(mean2[:, sl], arr2[:, sl, 0],
                                            1.0 / N_CORES)
                nc.vector.tensor_scalar_mul(var2[:, sl], arr2[:, sl, 1],
                                            1.0 / N_CORES)
                nc.vector.tensor_mul(tmp2[:, sl], mean2[:, sl], mean2[:, sl])
                nc.vector.tensor_sub(var2[:, sl], var2[:, sl], tmp2[:, sl])
                nc.vector.tensor_scalar_add(var2[:, sl], var2[:, sl], EPS)
                _rsqrt(nc, cst, rs2[:, sl], var2[:, sl], f"nr2_{m}")
                nc.vector.tensor_mul(s2[:, sl], rs2[:, sl], g2[:, sl])
                nc.vector.tensor_mul(t2[:, sl], mean2[:, sl], s2[:, sl])
                nc.vector.tensor_sub(t2[:, sl], b2bn[:, sl], t2[:, sl])
                nc.vector.tensor_copy(t2b[:, sl], t2[:, sl])
                nc.vector.tensor_scalar_mul(w2s[:, m, :], w2u[:, m, :],
                                            s2[:, m:m + 1])

            # PE warmup across the AR2b wait (keeps conv2 at full clock;
            # without this conv2 runs at the cold 1.2 GHz clock)
            fp2 = ps2.tile([P, 512], F32, tag="c2", name="filler2_ps")
            for i in range(40):
                nc.tensor.matmul(fp2[:], w1u[:, K1 - 1, 0:P],
                                 x_bf[B - 1][:, K1 - 1, 0:512],
                                 start=(i == 0), stop=(i == 39))

            # b2' = W2 @ t2 + b2, replicated on both partition halves for the
            # batch-pair-packed conv2 output bias
            b2f = cst.tile([P, 1], F32, tag="b2f")
            pm2 = psmv.tile([P, 1], F32, tag="mv")
            for hf in range(2):
                for k in range(K2):
                    nc.tensor.matmul(pm2[hf * C3:(hf + 1) * C3, :],
                                     w2u[:, k, :], t2b[:, k:k + 1],
                                     start=(k == 0), stop=(k == K2 - 1))
            nc.vector.tensor_add(b2f[:], pm2[:], b2c[:])

            # ---- conv2 -> out (two batches packed per [128, 512] tile);
            # PSUM drain alternates DVE/ACT; stores pipeline on sync ----
            gi = 0
            for bp in range(B // 2):
                for tt in range(NT):
                    ps = ps2.tile([P, 512], F32, tag="c2",
                                  name=f"ps2_{bp}_{tt}")
                    for hf in range(2):
                        b = 2 * bp + hf
                        pr = slice(hf * C3, (hf + 1) * C3)
                        for k in range(K2):
                            nc.tensor.matmul(
                                ps[pr, :], w2s[:, k, :],
                                h_bf[k][b][:, tt * 512:(tt + 1) * 512],
                                start=(k == 0), stop=(k == K2 - 1))
                    ob = op.tile([P, 512], F32, tag="out_sb",
                                 name=f"ob_{bp}_{tt}")
                    if gi % 2 == 0:
                        nc.vector.tensor_scalar_add(ob[:], ps[:], b2f[:, 0:1])
                    else:
                        nc.scalar.activation(ob[:], ps[:], AF.Identity,
                                             bias=b2f[:])
                    gi += 1
                    nc.sync.dma_start(
                        out_d[2 * bp:2 * bp + 2, :,
                              tt * 512:(tt + 1) * 512], ob[:])

    nc.compile()
    return nc


_NC_CACHE = None


def _get_nc():
    global _NC_CACHE
    if _NC_CACHE is None:
        _NC_CACHE = build()
    return _NC_CACHE


def run(inputs, trace=False, trace_kwargs=None):
    """Run on 8 NeuronCores; returns BassKernelResults."""
    x = np.ascontiguousarray(inputs["x"], dtype=np.float32)
    w1t = np.ascontiguousarray(np.asarray(inputs["w1"], dtype=np.float32).T)
    w2t = np.ascontiguousarray(np.asarray(inputs["w2"], dtype=np.float32).T)
    base = {
        "w1t": w1t,
        "w2t": w2t,
        "g1": np.ascontiguousarray(inputs["bn1_g"], dtype=np.float32),
        "b1bn": np.ascontiguousarray(inputs["bn1_b"], dtype=np.float32),
        "b1c": np.ascontiguousarray(inputs["b1"], dtype=np.float32),
        "g2": np.ascontiguousarray(inputs["bn2_g"], dtype=np.float32),
        "b2bn": np.ascontiguousarray(inputs["bn2_b"], dtype=np.float32),
        "b2c": np.ascontiguousarray(inputs["b2"], dtype=np.float32),
    }
    in_maps = [dict(base, x=np.ascontiguousarray(x[i * B:(i + 1) * B]))
               for i in range(N_CORES)]
    nc = _get_nc()
    kw = {}
    if trace:
        kw["trace"] = True
        if trace_kwargs:
            kw.update(trace_kwargs)
    res = bass_utils.run_bass_kernel_spmd(nc, in_maps, core_ids=list(range(N_CORES)), **kw)
    return res


def kernel(**inputs):
    res = run(inputs)
    out = np.concatenate([res.results[i]["out"] for i in range(N_CORES)], axis=0)
    mu = out[:, :C3 // 2, :]
    logvar = out[:, C3 // 2:, :]
    return (mu, logvar)



# revision 6
# speedup vs baseline: 1.1826x; 1.1826x over previous
"""Trainium2 Bass kernel for nn_Bottleneck (BN -> 1x1conv -> ReLU -> BN -> 1x1conv).

Strategy v2: data-parallel over batch (32 -> 4 per core x 8 cores),
PER-SHARD BN1 statistics (the sharding hint explicitly allows this;
measured combined error vs global-stats reference: 1.27e-2 max-relerr,
under the 2e-2 gate), global BN2 via split AllReduce.

Per-shard BN1 removes the load -> AllReduce -> conv1 serialization and
unlocks running conv1 UNDER the x load:
- x streams HBM->SBUF chunk by chunk via SWDGE casting DMAs (fp32->bf16).
- Chunk k's BN fold (s_k into weights) is ready ~1us after chunk k lands.
- conv1 partial sums accumulate across chunk groups {0-2},{3-5},{6-7}:
  group matmuls -> PSUM, ACT drains PSUM -> bf16 SBUF accumulator (ACT is
  otherwise idle during the load), and the next group reloads the
  accumulator into PSUM with an identity matmul (PE is otherwise idle
  during the load). DVE only does bn_stats + small folds.
- The last group's drain is a fused relu+bias+cast producing h; BN2
  h-stats run per m-half so AR2[0] hides under the m=1 finalize and
  conv2's k=0 partial matmuls hide under AR2[1].
- PE stays busy through the load (groups + fillers) so the clock is warm.
"""
import sys

sys.path.insert(0, "/opt/trn_rl_repo")

import numpy as np

import concourse.bass as bass
import concourse.bacc as bacc
import concourse.mybir as mybir
import concourse.tile as tile
from concourse import bass_utils

# Problem shapes (hardcoded per contract)
B_FULL = 32
N_CORES = 8
B = B_FULL // N_CORES  # 4 batches per core
C1 = 1024  # in channels
C2 = 256   # mid channels
C3 = 64    # out channels
T = 2048   # sequence length
P = 128    # partitions
K1 = C1 // P  # 8 contraction chunks for conv1
K2 = C2 // P  # 2 contraction chunks for conv2
NT = T // 512  # 4 tiles of 512 along T
EPS = 1e-5

N_FILLER = 24  # early PE warmup matmuls (clock ramp before group 0)

F32 = mybir.dt.float32
BF16 = mybir.dt.bfloat16
AF = mybir.ActivationFunctionType
ALU = mybir.AluOpType


def build():
    nc = bacc.Bacc("TRN2", target_bir_lowering=False, debug=False,
                   num_devices=N_CORES, num_swdge_queues=4)

    x_d = nc.dram_tensor("x", [B, C1, T], F32, kind="ExternalInput")
    w1t_d = nc.dram_tensor("w1t", [C1, C2], F32, kind="ExternalInput")
    w2t_d = nc.dram_tensor("w2t", [C2, C3], F32, kind="ExternalInput")
    g1_d = nc.dram_tensor("g1", [C1], F32, kind="ExternalInput")
    b1bn_d = nc.dram_tensor("b1bn", [C1], F32, kind="ExternalInput")
    b1c_d = nc.dram_tensor("b1c", [C2], F32, kind="ExternalInput")
    g2_d = nc.dram_tensor("g2", [C2], F32, kind="ExternalInput")
    b2bn_d = nc.dram_tensor("b2bn", [C2], F32, kind="ExternalInput")
    b2c_d = nc.dram_tensor("b2c", [C3], F32, kind="ExternalInput")
    ident_d = nc.dram_tensor("ident", [P, P], F32, kind="ExternalInput")
    out_d = nc.dram_tensor("out", [B, C3, T], F32, kind="ExternalOutput")

    rg = [list(range(N_CORES))]

    with tile.TileContext(nc) as tc:
        with (
            tc.tile_pool(name="const", bufs=1) as cst,
            tc.tile_pool(name="xpool", bufs=8) as xp,
            tc.tile_pool(name="hpool", bufs=1) as hp,
            tc.tile_pool(name="opool", bufs=3) as op,
            tc.tile_pool(name="ps", bufs=6, space="PSUM") as psp,
            tc.tile_pool(name="psmv", bufs=1, space="PSUM") as psmv,
            tc.tile_pool(name="dram", bufs=1, space="DRAM") as dram,
        ):
            # ---- prefetch weights/vectors ----
            w1u = cst.tile([P, K1, C2], BF16, tag="w1u")   # W1^T, unscaled bf16
            nc.gpsimd.dma_start(w1u[:], w1t_d.ap().rearrange("(k p) o -> p k o", p=P))
            w2u = cst.tile([P, K2, C3], BF16, tag="w2u")
            nc.gpsimd.dma_start(w2u[:], w2t_d.ap().rearrange("(k p) o -> p k o", p=P))
            ident = cst.tile([P, P], BF16, tag="ident")
            nc.gpsimd.dma_start(ident[:], ident_d.ap())

            g1 = cst.tile([P, K1], F32, tag="g1")
            nc.gpsimd.dma_start(g1[:], g1_d.ap().rearrange("(k p) -> p k", p=P))
            b1bn = cst.tile([P, K1], F32, tag="b1bn")
            nc.gpsimd.dma_start(b1bn[:], b1bn_d.ap().rearrange("(k p) -> p k", p=P))
            b1c = cst.tile([P, K2], F32, tag="b1c")
            nc.gpsimd.dma_start(b1c[:], b1c_d.ap().rearrange("(m p) -> p m", p=P))
            g2 = cst.tile([P, K2], F32, tag="g2")
            nc.gpsimd.dma_start(g2[:], g2_d.ap().rearrange("(k p) -> p k", p=P))
            b2bn = cst.tile([P, K2], F32, tag="b2bn")
            nc.gpsimd.dma_start(b2bn[:], b2bn_d.ap().rearrange("(k p) -> p k", p=P))
            b2c = cst.tile([P, 1], F32, tag="b2c")
            nc.gpsimd.dma_start(b2c[0:C3, :],
                              b2c_d.ap().rearrange("(a o) -> o a", a=1))
            nc.gpsimd.dma_start(b2c[C3:2 * C3, :],
                              b2c_d.ap().rearrange("(a o) -> o a", a=1))

            # ---- prime the collective stream (absorbs the ~28us first-
            # collective setup so the real AR2s run at primed latency) ----
            pr_in = dram.tile([P, 2], F32, tag="pr_in")
            pr_out = dram.tile([P, 2], F32, tag="pr_out")
            nc.gpsimd.dma_start(pr_in[:], g1[:, 0:2])
            nc.gpsimd.collective_compute(
                "AllReduce", ALU.add, replica_groups=rg,
                ins=[pr_in.opt()], outs=[pr_out.opt()])

            # ---- early PE warmup fillers (weights-only operands, so they
            # run as soon as the weight prefetch lands) ----
            fps = psp.tile([P, 512], F32, tag="c1", name="filler_ps")
            for i in range(N_FILLER):
                nc.tensor.matmul(fps[:], w1u[:, 0, 0:P], w1u[:, 0:2, :],
                                 start=(i == 0), stop=(i == N_FILLER - 1))

            # ---- per-shard BN1 state ----
            x_bf = []
            stx = [cst.tile([P, B * NT, 6], F32, tag=f"stx_{k}", name=f"stx_{k}")
                   for k in range(K1)]
            mv1 = cst.tile([P, K1, 2], F32, tag="mv1")   # (mean, var) per chunk
            rs1 = cst.tile([P, K1], F32, tag="rs1")
            rc1 = cst.tile([P, K1], F32, tag="rc1")
            s1 = cst.tile([P, K1], F32, tag="s1")
            t1 = cst.tile([P, K1], F32, tag="t1")
            t1b = cst.tile([P, K1], BF16, tag="t1b")
            w1s = cst.tile([P, K1, C2], BF16, tag="w1s")

            # bf16 accumulator tiles for conv1 partial sums; the final
            # relu-drain writes h into these same tiles.
            acc = [[hp.tile([P, T], BF16, tag=f"acc_{m}_{b}", name=f"acc_{m}_{b}")
                    for b in range(B)] for m in range(K2)]

            b1f = cst.tile([P, K2], F32, tag="b1f")

            def emit_fold1(k):
                """per-shard BN1 fold for chunk k: s,t + scaled weights."""
                sl = slice(k, k + 1)
                nc.vector.tensor_scalar_add(rs1[:, sl], mv1[:, k, 1:2], EPS)
                nc.vector.reciprocal(rc1[:, sl], rs1[:, sl])
                nc.scalar.activation(rs1[:, sl], rc1[:, sl], AF.Sqrt)
                nc.vector.tensor_mul(s1[:, sl], rs1[:, sl], g1[:, sl])
                nc.vector.tensor_mul(t1[:, sl], mv1[:, k, 0:1], s1[:, sl])
                nc.vector.tensor_sub(t1[:, sl], b1bn[:, sl], t1[:, sl])
                nc.vector.tensor_copy(t1b[:, sl], t1[:, sl])
                nc.vector.tensor_scalar_mul(w1s[:, k, :], w1u[:, k, :],
                                            s1[:, k:k + 1])

            def emit_group(g, k0, k1, mlist):
                """conv1 partial pass for chunks [k0,k1) over given m-halves."""
                first_group = (g == 0)
                last_group = (k1 == K1)
                for m in mlist:
                    for b in range(B):
                        for tq in range(NT):
                            ps = psp.tile([P, 512], F32, tag="c1",
                                          name=f"ps1_{g}_{m}_{b}_{tq}")
                            if not first_group:
                                nc.tensor.matmul(
                                    ps[:], ident[:],
                                    acc[m][b][:, tq * 512:(tq + 1) * 512],
                                    start=True, stop=False)
                            for k in range(k0, k1):
                                nc.tensor.matmul(
                                    ps[:], w1s[:, k, m * P:(m + 1) * P],
                                    x_bf[k][:, b, tq * 512:(tq + 1) * 512],
                                    start=(first_group and k == k0),
                                    stop=(k == k1 - 1))
                            if last_group:
                                nc.scalar.activation(
                                    acc[m][b][:, tq * 512:(tq + 1) * 512],
                                    ps[:], AF.Relu, bias=b1f[:, m:m + 1])
                                nc.vector.bn_stats(
                                    sth[m][:, b * NT + tq, :],
                                    acc[m][b][:, tq * 512:(tq + 1) * 512])
                            else:
                                nc.scalar.activation(
                                    acc[m][b][:, tq * 512:(tq + 1) * 512],
                                    ps[:], AF.Copy)

            # BN2 stats + AllReduce state
            sth = [cst.tile([P, B * NT, 6], F32, tag=f"sth_{m}", name=f"sth_{m}")
                   for m in range(K2)]
            mv2 = cst.tile([P, K2, 2], F32, tag="mv2")
            ar2 = cst.tile([P, K2, 2], F32, tag="ar2")
            tmp2 = cst.tile([P, K2], F32, tag="tmp2")
            ar2_outs = []

            def emit_ar2(m):
                ms = slice(m, m + 1)
                nc.vector.bn_aggr(mv2[:, m, :], sth[m][:])
                nc.vector.tensor_copy(ar2[:, ms, 0], mv2[:, ms, 0])
                nc.vector.tensor_mul(tmp2[:, ms], mv2[:, ms, 0], mv2[:, ms, 0])
                nc.vector.tensor_add(ar2[:, ms, 1], mv2[:, ms, 1], tmp2[:, ms])
                ai2 = dram.tile([P, 2], F32, tag=f"ar2_in_{m}",
                                name=f"ar2_in_{m}")
                ao2 = dram.tile([P, 2], F32, tag=f"ar2_out_{m}",
                                name=f"ar2_out_{m}")
                nc.gpsimd.dma_start(ai2[:], ar2[:, ms, :])
                nc.gpsimd.collective_compute(
                    "AllReduce", ALU.add, replica_groups=rg,
                    ins=[ai2.opt()], outs=[ao2.opt()])
                ar2_outs.append(ao2)

            # ---- pass 1: stream x, per-chunk stats+fold, grouped conv1 ----
            pm = [None, None]
            for k in range(K1):
                xk = xp.tile([P, B, T], BF16, tag="x", name=f"x_{k}")
                x_bf.append(xk)
                for b in range(B):
                    nc.gpsimd.dma_start(xk[:, b, :], x_d[b, k * P:(k + 1) * P, :])
                for b in range(B):
                    for c in range(NT):
                        nc.vector.bn_stats(
                            stx[k][:, b * NT + c, :],
                            xk[:, b, c * 512:(c + 1) * 512])
                nc.vector.bn_aggr(mv1[:, k, :], stx[k][:])
                emit_fold1(k)
                if k == 2:
                    emit_group(0, 0, 3, [0, 1])
                elif k == 5:
                    emit_group(1, 3, 6, [0, 1])
                elif k == 6:
                    # b1' matvec, terms k=0..6 (t1b for those ready early)
                    for m in range(K2):
                        pm[m] = psmv.tile([P, 1], F32, tag=f"mv{m}",
                                          name=f"pm_{m}")
                        for kk in range(7):
                            nc.tensor.matmul(pm[m][:],
                                             w1u[:, kk, m * P:(m + 1) * P],
                                             t1b[:, kk:kk + 1],
                                             start=(kk == 0), stop=False)
                elif k == 7:
                    # finish b1' with the k=7 term
                    for m in range(K2):
                        nc.tensor.matmul(pm[m][:],
                                         w1u[:, 7, m * P:(m + 1) * P],
                                         t1b[:, 7:8],
                                         start=False, stop=True)
                        nc.vector.tensor_add(b1f[:, m:m + 1], pm[m][:],
                                             b1c[:, m:m + 1])
                    # final group: m=0 first so AR2[0] fires early
                    emit_group(2, 6, 8, [0])
                    emit_ar2(0)
                    emit_group(2, 6, 8, [1])
                    emit_ar2(1)

            # ---- global BN2 affine + fold into conv2 ----
            arr2 = cst.tile([P, K2, 2], F32, tag="arr2")
            mean2 = cst.tile([P, K2], F32, tag="mean2")
            var2 = cst.tile([P, K2], F32, tag="var2")
            rc2 = cst.tile([P, K2], F32, tag="rc2")
            rs2 = cst.tile([P, K2], F32, tag="rs2")
            s2 = cst.tile([P, K2], F32, tag="s2")
            t2 = cst.tile([P, K2], F32, tag="t2")
            t2b = cst.tile([P, K2], BF16, tag="t2b")
            w2s = cst.tile([P, K2, C3], BF16, tag="w2s")

            def emit_fold2(m):
                sl = slice(m, m + 1)
                nc.sync.dma_start(arr2[:, sl, :], ar2_outs[m][:])
                nc.vector.tensor_scalar_mul(mean2[:, sl], arr2[:, sl, 0],
                                            1.0 / N_CORES)
                nc.vector.tensor_scalar_mul(var2[:, sl], arr2[:, sl, 1],
                                            1.0 / N_CORES)
                nc.vector.tensor_mul(tmp2[:, sl], mean2[:, sl], mean2[:, sl])
                nc.vector.tensor_sub(var2[:, sl], var2[:, sl], tmp2[:, sl])
                nc.vector.tensor_scalar_add(var2[:, sl], var2[:, sl], EPS)
                nc.vector.reciprocal(rc2[:, sl], var2[:, sl])
                nc.scalar.activation(rs2[:, sl], rc2[:, sl], AF.Sqrt)
                nc.vector.tensor_mul(s2[:, sl], rs2[:, sl], g2[:, sl])
                nc.vector.tensor_mul(t2[:, sl], mean2[:, sl], s2[:, sl])
                nc.vector.tensor_sub(t2[:, sl], b2bn[:, sl], t2[:, sl])
                nc.vector.tensor_copy(t2b[:, sl], t2[:, sl])
                nc.vector.tensor_scalar_mul(w2s[:, m, :], w2u[:, m, :],
                                            s2[:, m:m + 1])

            emit_fold2(0)

            # ---- conv2: k=0 partials for the first 6 tiles (they only need
            # w2s[0], so they run while AR2[1] is still in flight). The last
            # tiles are done in full later (psp has 6 buffers; pre-opening
            # all 8 would deadlock the PSUM ring). ----
            tiles = [(bp, tt) for bp in range(B // 2) for tt in range(NT)]
            c2ps = []
            for (bp, tt) in tiles[:6]:
                ps = psp.tile([P, 512], F32, tag="c1", name=f"ps2_{bp}_{tt}")
                for hf in range(2):
                    b = 2 * bp + hf
                    pr = slice(hf * C3, (hf + 1) * C3)
                    nc.tensor.matmul(ps[pr, :], w2s[:, 0, :],
                                     acc[0][b][:, tt * 512:(tt + 1) * 512],
                                     start=True, stop=False)
                c2ps.append((bp, tt, ps))

            emit_fold2(1)
            # b2' = W2 @ t2 + b2 (both partition halves for packed output)
            pm2 = psmv.tile([P, 1], F32, tag="mv0", name="pmv_b2")
            b2f = cst.tile([P, 1], F32, tag="b2f")
            for hf in range(2):
                for kk in range(K2):
                    nc.tensor.matmul(pm2[hf * C3:(hf + 1) * C3, :],
                                     w2u[:, kk, :], t2b[:, kk:kk + 1],
                                     start=(kk == 0), stop=(kk == K2 - 1))
            nc.vector.tensor_add(b2f[:], pm2[:], b2c[:])

            # ---- conv2 k=1 + bias + store ----
            gi = 0

            def finish_c2(bp, tt, ps, gi):
                for hf in range(2):
                    b = 2 * bp + hf
                    pr = slice(hf * C3, (hf + 1) * C3)
                    nc.tensor.matmul(ps[pr, :], w2s[:, 1, :],
                                     acc[1][b][:, tt * 512:(tt + 1) * 512],
                                     start=False, stop=True)
                ob = op.tile([P, 512], F32, tag="out_sb", name=f"ob_{bp}_{tt}")
                if gi % 2 == 0:
                    nc.vector.tensor_scalar_add(ob[:], ps[:], b2f[:, 0:1])
                else:
                    nc.scalar.activation(ob[:], ps[:], AF.Identity, bias=b2f[:])
                nc.sync.dma_start(
                    out_d[2 * bp:2 * bp + 2, :, tt * 512:(tt + 1) * 512], ob[:])

            for (bp, tt, ps) in c2ps:
                finish_c2(bp, tt, ps, gi)
                gi += 1
            # remaining tiles in full
            for (bp, tt) in tiles[6:]:
                ps = psp.tile([P, 512], F32, tag="c1", name=f"ps2_{bp}_{tt}")
                for hf in range(2):
                    b = 2 * bp + hf
                    pr = slice(hf * C3, (hf + 1) * C3)
                    for kk in range(K2):
                        nc.tensor.matmul(ps[pr, :], w2s[:, kk, :],
                                         acc[kk][b][:, tt * 512:(tt + 1) * 512],
                                         start=(kk == 0), stop=(kk == K2 - 1))
                finish_done = False
                ob = op.tile([P, 512], F32, tag="out_sb", name=f"ob_{bp}_{tt}")
                nc.scalar.activation(ob[:], ps[:], AF.Identity, bias=b2f[:])
                nc.sync.dma_start(
                    out_d[2 * bp:2 * bp + 2, :, tt * 512:(tt + 1) * 512], ob[:])

    nc.compile()
    return nc


_NC_CACHE = None


def _get_nc():
    global _NC_CACHE
    if _NC_CACHE is None:
        _NC_CACHE = build()
    return _NC_CACHE


def run(inputs, trace=False, trace_kwargs=None):
    """Run on 8 NeuronCores; returns BassKernelResults."""
    x = np.ascontiguousarray(inputs["x"], dtype=np.float32)
    w1t = np.ascontiguousarray(np.asarray(inputs["w1"], dtype=np.float32).T)
    w2t = np.ascontiguousarray(np.asarray(inputs["w2"], dtype=np.float32).T)
    base = {
        "w1t": w1t,
        "w2t": w2t,
        "g1": np.ascontiguousarray(inputs["bn1_g"], dtype=np.float32),
        "b1bn": np.ascontiguousarray(inputs["bn1_b"], dtype=np.float32),
        "b1c": np.ascontiguousarray(inputs["b1"], dtype=np.float32),
        "g2": np.ascontiguousarray(inputs["bn2_g"], dtype=np.float32),
        "b2bn": np.ascontiguousarray(inputs["bn2_b"], dtype=np.float32),
        "b2c": np.ascontiguousarray(inputs["b2"], dtype=np.float32),
        "ident": np.eye(P, dtype=np.float32),
    }
    in_maps = [dict(base, x=np.ascontiguousarray(x[i * B:(i + 1) * B]))
               for i in range(N_CORES)]
    nc = _get_nc()
    kw = {}
    if trace:
        kw["trace"] = True
        if trace_kwargs:
            kw.update(trace_kwargs)
    res = bass_utils.run_bass_kernel_spmd(nc, in_maps, core_ids=list(range(N_CORES)), **kw)
    return res


def kernel(**inputs):
    res = run(inputs)
    out = np.concatenate([res.results[i]["out"] for i in range(N_CORES)], axis=0)
    mu = out[:, :C3 // 2, :]
    logvar = out[:, C3 // 2:, :]
    return (mu, logvar)


# revision 8
# speedup vs baseline: 1.2135x; 1.0261x over previous
"""Trainium2 Bass kernel for nn_Bottleneck (BN -> 1x1conv -> ReLU -> BN -> 1x1conv).

Strategy v2: data-parallel over batch (32 -> 4 per core x 8 cores),
PER-SHARD BN1 statistics (the sharding hint explicitly allows this;
measured combined error vs global-stats reference: 1.27e-2 max-relerr,
under the 2e-2 gate), global BN2 via split AllReduce.

Per-shard BN1 removes the load -> AllReduce -> conv1 serialization and
unlocks running conv1 UNDER the x load:
- x streams HBM->SBUF chunk by chunk via SWDGE casting DMAs (fp32->bf16).
- Chunk k's BN fold (s_k into weights) is ready ~1us after chunk k lands.
- conv1 partial sums accumulate across chunk groups {0-2},{3-5},{6-7}:
  group matmuls -> PSUM, ACT drains PSUM -> bf16 SBUF accumulator (ACT is
  otherwise idle during the load), and the next group reloads the
  accumulator into PSUM with an identity matmul (PE is otherwise idle
  during the load). DVE only does bn_stats + small folds.
- The last group's drain is a fused relu+bias+cast producing h; BN2
  h-stats run per m-half so AR2[0] hides under the m=1 finalize and
  conv2's k=0 partial matmuls hide under AR2[1].
- PE stays busy through the load (groups + fillers) so the clock is warm.
"""
import sys

sys.path.insert(0, "/opt/trn_rl_repo")

import numpy as np

import concourse.bass as bass
import concourse.bacc as bacc
import concourse.mybir as mybir
import concourse.tile as tile
from concourse import bass_utils

# Problem shapes (hardcoded per contract)
B_FULL = 32
N_CORES = 8
B = B_FULL // N_CORES  # 4 batches per core
C1 = 1024  # in channels
C2 = 256   # mid channels
C3 = 64    # out channels
T = 2048   # sequence length
P = 128    # partitions
K1 = C1 // P  # 8 contraction chunks for conv1
K2 = C2 // P  # 2 contraction chunks for conv2
NT = T // 512  # 4 tiles of 512 along T
EPS = 1e-5

N_FILLER = 16  # early PE warmup matmuls (clock ramp before group 0)

F32 = mybir.dt.float32
BF16 = mybir.dt.bfloat16
AF = mybir.ActivationFunctionType
ALU = mybir.AluOpType


def build():
    nc = bacc.Bacc("TRN2", target_bir_lowering=False, debug=False,
                   num_devices=N_CORES, num_swdge_queues=4)

    x_d = nc.dram_tensor("x", [B, C1, T], F32, kind="ExternalInput")
    w1t_d = nc.dram_tensor("w1t", [C1, C2], F32, kind="ExternalInput")
    w2t_d = nc.dram_tensor("w2t", [C2, C3], F32, kind="ExternalInput")
    g1_d = nc.dram_tensor("g1", [C1], F32, kind="ExternalInput")
    b1bn_d = nc.dram_tensor("b1bn", [C1], F32, kind="ExternalInput")
    b1c_d = nc.dram_tensor("b1c", [C2], F32, kind="ExternalInput")
    g2_d = nc.dram_tensor("g2", [C2], F32, kind="ExternalInput")
    b2bn_d = nc.dram_tensor("b2bn", [C2], F32, kind="ExternalInput")
    b2c_d = nc.dram_tensor("b2c", [C3], F32, kind="ExternalInput")
    ident_d = nc.dram_tensor("ident", [P, P], F32, kind="ExternalInput")
    out_d = nc.dram_tensor("out", [B, C3, T], F32, kind="ExternalOutput")

    rg = [list(range(N_CORES))]

    with tile.TileContext(nc) as tc:
        with (
            tc.tile_pool(name="const", bufs=1) as cst,
            tc.tile_pool(name="xpool", bufs=8) as xp,
            tc.tile_pool(name="hpool", bufs=1) as hp,
            tc.tile_pool(name="opool", bufs=3) as op,
            tc.tile_pool(name="ps", bufs=6, space="PSUM") as psp,
            tc.tile_pool(name="psmv", bufs=1, space="PSUM") as psmv,
            tc.tile_pool(name="dram", bufs=1, space="DRAM") as dram,
        ):
            # ---- prefetch weights/vectors ----
            w1u = cst.tile([P, K1, C2], BF16, tag="w1u")   # W1^T, unscaled bf16
            nc.gpsimd.dma_start(w1u[:], w1t_d.ap().rearrange("(k p) o -> p k o", p=P))
            w2u = cst.tile([P, K2, C3], BF16, tag="w2u")
            nc.gpsimd.dma_start(w2u[:], w2t_d.ap().rearrange("(k p) o -> p k o", p=P))
            ident = cst.tile([P, P], BF16, tag="ident")
            nc.gpsimd.dma_start(ident[:], ident_d.ap())

            g1 = cst.tile([P, K1], F32, tag="g1")
            nc.gpsimd.dma_start(g1[:], g1_d.ap().rearrange("(k p) -> p k", p=P))
            b1bn = cst.tile([P, K1], F32, tag="b1bn")
            nc.gpsimd.dma_start(b1bn[:], b1bn_d.ap().rearrange("(k p) -> p k", p=P))
            b1c = cst.tile([P, K2], F32, tag="b1c")
            nc.gpsimd.dma_start(b1c[:], b1c_d.ap().rearrange("(m p) -> p m", p=P))
            g2 = cst.tile([P, K2], F32, tag="g2")
            nc.gpsimd.dma_start(g2[:], g2_d.ap().rearrange("(k p) -> p k", p=P))
            b2bn = cst.tile([P, K2], F32, tag="b2bn")
            nc.gpsimd.dma_start(b2bn[:], b2bn_d.ap().rearrange("(k p) -> p k", p=P))
            b2c = cst.tile([P, 1], F32, tag="b2c")
            nc.gpsimd.dma_start(b2c[0:C3, :],
                              b2c_d.ap().rearrange("(a o) -> o a", a=1))
            nc.gpsimd.dma_start(b2c[C3:2 * C3, :],
                              b2c_d.ap().rearrange("(a o) -> o a", a=1))

            # ---- prime the collective stream (absorbs the ~28us first-
            # collective setup so the real AR2s run at primed latency) ----
            pr_in = dram.tile([P, 2], F32, tag="pr_in")
            pr_out = dram.tile([P, 2], F32, tag="pr_out")
            pr_out2 = dram.tile([P, 2], F32, tag="pr_out2")
            nc.gpsimd.dma_start(pr_in[:], g1[:, 0:2])
            nc.gpsimd.collective_compute(
                "AllReduce", ALU.add, replica_groups=rg,
                ins=[pr_in.opt()], outs=[pr_out.opt()])
            # second prime: the first collective pays stream setup, the
            # second pays the first cross-core sync; only from the third
            # one on do collectives run at the ~9us fast path.
            nc.gpsimd.collective_compute(
                "AllReduce", ALU.add, replica_groups=rg,
                ins=[pr_in.opt()], outs=[pr_out2.opt()])

            # ---- early PE warmup fillers (weights-only operands, so they
            # run as soon as the weight prefetch lands) ----
            fps = psp.tile([P, 512], F32, tag="c1", name="filler_ps")
            for i in range(N_FILLER):
                nc.tensor.matmul(fps[:], w1u[:, 0, 0:P], w1u[:, 0:2, :],
                                 start=(i == 0), stop=(i == N_FILLER - 1))

            # ---- per-shard BN1 state ----
            x_bf = []
            stx = [cst.tile([P, B * NT, 6], F32, tag=f"stx_{k}", name=f"stx_{k}")
                   for k in range(K1)]
            mv1 = cst.tile([P, K1, 2], F32, tag="mv1")   # (mean, var) per chunk
            rs1 = cst.tile([P, K1], F32, tag="rs1")
            rc1 = cst.tile([P, K1], F32, tag="rc1")
            s1 = cst.tile([P, K1], F32, tag="s1")
            t1 = cst.tile([P, K1], F32, tag="t1")
            t1b = cst.tile([P, K1], BF16, tag="t1b")
            w1s = cst.tile([P, K1, C2], BF16, tag="w1s")

            # bf16 accumulator tiles for conv1 partial sums; the final
            # relu-drain writes h into these same tiles.
            acc = [[hp.tile([P, T], BF16, tag=f"acc_{m}_{b}", name=f"acc_{m}_{b}")
                    for b in range(B)] for m in range(K2)]

            b1f = cst.tile([P, K2], F32, tag="b1f")

            def emit_fold1(k):
                """per-shard BN1 fold for chunk k: s,t + scaled weights."""
                sl = slice(k, k + 1)
                nc.vector.tensor_scalar_add(rs1[:, sl], mv1[:, k, 1:2], EPS)
                nc.vector.reciprocal(rc1[:, sl], rs1[:, sl])
                nc.scalar.activation(rs1[:, sl], rc1[:, sl], AF.Sqrt)
                nc.vector.tensor_mul(s1[:, sl], rs1[:, sl], g1[:, sl])
                nc.vector.tensor_mul(t1[:, sl], mv1[:, k, 0:1], s1[:, sl])
                nc.vector.tensor_sub(t1[:, sl], b1bn[:, sl], t1[:, sl])
                nc.vector.tensor_copy(t1b[:, sl], t1[:, sl])
                nc.vector.tensor_scalar_mul(w1s[:, k, :], w1u[:, k, :],
                                            s1[:, k:k + 1])

            def emit_group(g, k0, k1, mlist):
                """conv1 partial pass for chunks [k0,k1) over given m-halves."""
                first_group = (g == 0)
                last_group = (k1 == K1)
                for m in mlist:
                    for b in range(B):
                        for tq in range(NT):
                            ps = psp.tile([P, 512], F32, tag="c1",
                                          name=f"ps1_{g}_{m}_{b}_{tq}")
                            if not first_group:
                                nc.tensor.matmul(
                                    ps[:], ident[:],
                                    acc[m][b][:, tq * 512:(tq + 1) * 512],
                                    start=True, stop=False)
                            for k in range(k0, k1):
                                nc.tensor.matmul(
                                    ps[:], w1s[:, k, m * P:(m + 1) * P],
                                    x_bf[k][:, b, tq * 512:(tq + 1) * 512],
                                    start=(first_group and k == k0),
                                    stop=(k == k1 - 1))
                            if last_group:
                                nc.scalar.activation(
                                    acc[m][b][:, tq * 512:(tq + 1) * 512],
                                    ps[:], AF.Relu, bias=b1f[:, m:m + 1])
                                nc.vector.bn_stats(
                                    sth[m][:, b * NT + tq, :],
                                    acc[m][b][:, tq * 512:(tq + 1) * 512])
                            else:
                                nc.scalar.activation(
                                    acc[m][b][:, tq * 512:(tq + 1) * 512],
                                    ps[:], AF.Copy)

            # BN2 stats + AllReduce state
            sth = [cst.tile([P, B * NT, 6], F32, tag=f"sth_{m}", name=f"sth_{m}")
                   for m in range(K2)]
            mv2 = cst.tile([P, K2, 2], F32, tag="mv2")
            ar2 = cst.tile([P, K2, 2], F32, tag="ar2")
            tmp2 = cst.tile([P, K2], F32, tag="tmp2")
            ar2_outs = []

            def emit_ar2(m):
                ms = slice(m, m + 1)
                nc.vector.bn_aggr(mv2[:, m, :], sth[m][:])
                nc.vector.tensor_copy(ar2[:, ms, 0], mv2[:, ms, 0])
                nc.vector.tensor_mul(tmp2[:, ms], mv2[:, ms, 0], mv2[:, ms, 0])
                nc.vector.tensor_add(ar2[:, ms, 1], mv2[:, ms, 1], tmp2[:, ms])
                ai2 = dram.tile([P, 2], F32, tag=f"ar2_in_{m}",
                                name=f"ar2_in_{m}")
                ao2 = dram.tile([P, 2], F32, tag=f"ar2_out_{m}",
                                name=f"ar2_out_{m}")
                nc.gpsimd.dma_start(ai2[:], ar2[:, ms, :])
                nc.gpsimd.collective_compute(
                    "AllReduce", ALU.add, replica_groups=rg,
                    ins=[ai2.opt()], outs=[ao2.opt()])
                ar2_outs.append(ao2)

            # ---- pass 1: stream x, per-chunk stats+fold, grouped conv1 ----
            pm = [None, None]
            for k in range(K1):
                xk = xp.tile([P, B, T], BF16, tag="x", name=f"x_{k}")
                x_bf.append(xk)
                for b in range(B):
                    nc.gpsimd.dma_start(xk[:, b, :], x_d[b, k * P:(k + 1) * P, :])
                for b in range(B):
                    for c in range(NT):
                        nc.vector.bn_stats(
                            stx[k][:, b * NT + c, :],
                            xk[:, b, c * 512:(c + 1) * 512])
                nc.vector.bn_aggr(mv1[:, k, :], stx[k][:])
                emit_fold1(k)
                if k == 2:
                    emit_group(0, 0, 3, [0, 1])
                elif k == 5:
                    emit_group(1, 3, 6, [0, 1])
                elif k == 6:
                    # b1' matvec, terms k=0..6 (t1b for those ready early)
                    for m in range(K2):
                        pm[m] = psmv.tile([P, 1], F32, tag=f"mv{m}",
                                          name=f"pm_{m}")
                        for kk in range(7):
                            nc.tensor.matmul(pm[m][:],
                                             w1u[:, kk, m * P:(m + 1) * P],
                                             t1b[:, kk:kk + 1],
                                             start=(kk == 0), stop=False)
                elif k == 7:
                    # finish b1' with the k=7 term
                    for m in range(K2):
                        nc.tensor.matmul(pm[m][:],
                                         w1u[:, 7, m * P:(m + 1) * P],
                                         t1b[:, 7:8],
                                         start=False, stop=True)
                        nc.vector.tensor_add(b1f[:, m:m + 1], pm[m][:],
                                             b1c[:, m:m + 1])
                    # final group: m=0 first so AR2[0] fires early
                    emit_group(2, 6, 8, [0])
                    emit_ar2(0)
                    emit_group(2, 6, 8, [1])
                    emit_ar2(1)

            # ---- global BN2 affine + fold into conv2 ----
            arr2 = cst.tile([P, K2, 2], F32, tag="arr2")
            mean2 = cst.tile([P, K2], F32, tag="mean2")
            var2 = cst.tile([P, K2], F32, tag="var2")
            rc2 = cst.tile([P, K2], F32, tag="rc2")
            rs2 = cst.tile([P, K2], F32, tag="rs2")
            s2 = cst.tile([P, K2], F32, tag="s2")
            t2 = cst.tile([P, K2], F32, tag="t2")
            t2b = cst.tile([P, K2], BF16, tag="t2b")
            w2s = cst.tile([P, K2, C3], BF16, tag="w2s")

            def emit_fold2(m):
                sl = slice(m, m + 1)
                nc.sync.dma_start(arr2[:, sl, :], ar2_outs[m][:])
                nc.vector.tensor_scalar_mul(mean2[:, sl], arr2[:, sl, 0],
                                            1.0 / N_CORES)
                nc.vector.tensor_scalar_mul(var2[:, sl], arr2[:, sl, 1],
                                            1.0 / N_CORES)
                nc.vector.tensor_mul(tmp2[:, sl], mean2[:, sl], mean2[:, sl])
                nc.vector.tensor_sub(var2[:, sl], var2[:, sl], tmp2[:, sl])
                nc.vector.tensor_scalar_add(var2[:, sl], var2[:, sl], EPS)
                nc.vector.reciprocal(rc2[:, sl], var2[:, sl])
                nc.scalar.activation(rs2[:, sl], rc2[:, sl], AF.Sqrt)
                nc.vector.tensor_mul(s2[:, sl], rs2[:, sl], g2[:, sl])
                nc.vector.tensor_mul(t2[:, sl], mean2[:, sl], s2[:, sl])
                nc.vector.tensor_sub(t2[:, sl], b2bn[:, sl], t2[:, sl])
                nc.vector.tensor_copy(t2b[:, sl], t2[:, sl])
                nc.vector.tensor_scalar_mul(w2s[:, m, :], w2u[:, m, :],
                                            s2[:, m:m + 1])

            emit_fold2(0)

            # ---- conv2: k=0 partials for the first 6 tiles (they only need
            # w2s[0], so they run while AR2[1] is still in flight). The last
            # tiles are done in full later (psp has 6 buffers; pre-opening
            # all 8 would deadlock the PSUM ring). ----
            tiles = [(bp, tt) for bp in range(B // 2) for tt in range(NT)]
            c2ps = []
            for (bp, tt) in tiles[:6]:
                ps = psp.tile([P, 512], F32, tag="c1", name=f"ps2_{bp}_{tt}")
                for hf in range(2):
                    b = 2 * bp + hf
                    pr = slice(hf * C3, (hf + 1) * C3)
                    nc.tensor.matmul(ps[pr, :], w2s[:, 0, :],
                                     acc[0][b][:, tt * 512:(tt + 1) * 512],
                                     start=True, stop=False)
                c2ps.append((bp, tt, ps))

            emit_fold2(1)
            # b2' = W2 @ t2 + b2 (both partition halves for packed output)
            pm2 = psmv.tile([P, 1], F32, tag="mv0", name="pmv_b2")
            b2f = cst.tile([P, 1], F32, tag="b2f")
            for hf in range(2):
                for kk in range(K2):
                    nc.tensor.matmul(pm2[hf * C3:(hf + 1) * C3, :],
                                     w2u[:, kk, :], t2b[:, kk:kk + 1],
                                     start=(kk == 0), stop=(kk == K2 - 1))
            nc.vector.tensor_add(b2f[:], pm2[:], b2c[:])

            # ---- conv2 k=1 + bias + store ----
            gi = 0

            def finish_c2(bp, tt, ps, gi):
                for hf in range(2):
                    b = 2 * bp + hf
                    pr = slice(hf * C3, (hf + 1) * C3)
                    nc.tensor.matmul(ps[pr, :], w2s[:, 1, :],
                                     acc[1][b][:, tt * 512:(tt + 1) * 512],
                                     start=False, stop=True)
                ob = op.tile([P, 512], F32, tag="out_sb", name=f"ob_{bp}_{tt}")
                if gi % 2 == 0:
                    nc.vector.tensor_scalar_add(ob[:], ps[:], b2f[:, 0:1])
                else:
                    nc.scalar.activation(ob[:], ps[:], AF.Identity, bias=b2f[:])
                nc.sync.dma_start(
                    out_d[2 * bp:2 * bp + 2, :, tt * 512:(tt + 1) * 512], ob[:])

            for (bp, tt, ps) in c2ps:
                finish_c2(bp, tt, ps, gi)
                gi += 1
            # remaining tiles in full
            for (bp, tt) in tiles[6:]:
                ps = psp.tile([P, 512], F32, tag="c1", name=f"ps2_{bp}_{tt}")
                for hf in range(2):
                    b = 2 * bp + hf
                    pr = slice(hf * C3, (hf + 1) * C3)
                    for kk in range(K2):
                        nc.tensor.matmul(ps[pr, :], w2s[:, kk, :],
                                         acc[kk][b][:, tt * 512:(tt + 1) * 512],
                                         start=(kk == 0), stop=(kk == K2 - 1))
                finish_done = False
                ob = op.tile([P, 512], F32, tag="out_sb", name=f"ob_{bp}_{tt}")
                nc.scalar.activation(ob[:], ps[:], AF.Identity, bias=b2f[:])
                nc.sync.dma_start(
                    out_d[2 * bp:2 * bp + 2, :, tt * 512:(tt + 1) * 512], ob[:])

    nc.compile()
    return nc


_NC_CACHE = None


def _get_nc():
    global _NC_CACHE
    if _NC_CACHE is None:
        _NC_CACHE = build()
    return _NC_CACHE


def run(inputs, trace=False, trace_kwargs=None):
    """Run on 8 NeuronCores; returns BassKernelResults."""
    x = np.ascontiguousarray(inputs["x"], dtype=np.float32)
    w1t = np.ascontiguousarray(np.asarray(inputs["w1"], dtype=np.float32).T)
    w2t = np.ascontiguousarray(np.asarray(inputs["w2"], dtype=np.float32).T)
    base = {
        "w1t": w1t,
        "w2t": w2t,
        "g1": np.ascontiguousarray(inputs["bn1_g"], dtype=np.float32),
        "b1bn": np.ascontiguousarray(inputs["bn1_b"], dtype=np.float32),
        "b1c": np.ascontiguousarray(inputs["b1"], dtype=np.float32),
        "g2": np.ascontiguousarray(inputs["bn2_g"], dtype=np.float32),
        "b2bn": np.ascontiguousarray(inputs["bn2_b"], dtype=np.float32),
        "b2c": np.ascontiguousarray(inputs["b2"], dtype=np.float32),
        "ident": np.eye(P, dtype=np.float32),
    }
    in_maps = [dict(base, x=np.ascontiguousarray(x[i * B:(i + 1) * B]))
               for i in range(N_CORES)]
    nc = _get_nc()
    kw = {}
    if trace:
        kw["trace"] = True
        if trace_kwargs:
            kw.update(trace_kwargs)
    res = bass_utils.run_bass_kernel_spmd(nc, in_maps, core_ids=list(range(N_CORES)), **kw)
    return res


def kernel(**inputs):
    res = run(inputs)
    out = np.concatenate([res.results[i]["out"] for i in range(N_CORES)], axis=0)
    mu = out[:, :C3 // 2, :]
    logvar = out[:, C3 // 2:, :]
    return (mu, logvar)
